# revision 17
# baseline (speedup 1.0000x reference)
"""BENDR contrastive-loss kernel for Trainium2 (8 NeuronCores).

Reference computation (see problem): for each (b, t):
  logits[b*T+t, 0]   = cos(z[b,:,t], c[b,:,t+1]) / TEMP
  logits[b*T+t, 1+k] = cos(z[b,:,t], z[b,:,n(b,t,k)]) / TEMP
with n(b,t,k) = negative_inds[b, t*K+k] (row-local), TEMP=0.5.

Strategy: data-parallel over batch (2 rows per core).  Every negative logit
is an entry of the symmetric Gram matrix G = z^T z (z columns = feature
vectors), scaled by 2/(|z_t||z_j|); the norms are G's own diagonal.  So the
device only computes, per batch row:
  - the UPPER-TRIANGLE 128-row blocks of G (raw bf16 z, f32 PSUM) -> fp16
    (tau-th block covers columns [128*tau, T), so ~half the matmuls and
    traffic of the full Gram; the host mirrors lower-triangle lookups),
  - u[t]   = sum_f z[f,t]*c[f,t]   (DVE mult + ones-matmul reduction),
  - nc2[t] = sum_f c[f,t]^2        (same),
shipped as one [1, T] f32 DMA straight out of PSUM partition 0.
The host (pure indexing + O(output) normalize, same spirit as the
baseline's host gather) forms
  neg = 2*G[t,n] / sqrt(G[t,t]*G[n,n]),  pos = 2*u[t] / sqrt(G[t,t]*nc2[t]).

vs. the previous full-Gram kernel this removes the entire on-device
normalization pipeline (reciprocal 62us, input casts, scale mults, scaled
copies) whose DVE/ACT bursts head-blocked PSUM evacuation and let the PE's
HAM clock-gate throttle it to 1.2 GHz.  Here DVE/ACT only carry light
elementwise work + evacuation, and the PE stream is dense.

The gather itself stays on host: GPSIMD indirect_copy measures ~29us per
1024 indices and indirect DMA ~62ns/row -- computing the Gram block on the
PE and shipping fp16 is far cheaper than any on-device gather.
"""

import sys

for _p in ("/opt/trn_rl_repo",):
    if _p not in sys.path:
        sys.path.append(_p)

import numpy as np
import ml_dtypes

import concourse.bass as bass
import concourse.mybir as mybir
from concourse import tile as _tile
from concourse.tile import TileContext
from concourse.bass_utils import run_bass_kernel_spmd

dt = mybir.dt


B, F, T, K = 16, 256, 2048, 20
NCORES = 8
ROWS = B // NCORES          # batch rows per core
NBLK = T // 128             # t-blocks per batch row
FCH = F // 128              # f chunks (partition dim)
EPS = 1e-8

# ---------------------------------------------------------------------------
# Walrus in this container rejects instructions that carry more than one
# semaphore wait ("Too many sync wait commands").  Two shims fix that: the
# tile tail drain gets its waits on single-wait NOPs, and a post-pass splits
# any remaining multi-wait instruction.
# ---------------------------------------------------------------------------


def _patched_drain_and_barrier(self, tick_clock, wait_clock):
    nop0 = self.nc.sync.nop(nofuse=True, hint="tail_wait")
    wait_clock.add_sem_waits(
        nop0.ins, _tile.ScopedClock({None: tick_clock.global_clock})
    )
    si = nop0.ins.sync_info
    if si is not None and len(si.on_wait) > 1:
        waits = list(si.on_wait)
        nop0.ins.sync_info = mybir.SyncInfo(
            on_wait=waits[:1], on_update=list(si.on_update)
        )
        for w in waits[1:]:
            nopi = self.nc.sync.nop(nofuse=True, hint="tail_wait")
            nopi.ins.sync_info = mybir.SyncInfo(on_wait=[w], on_update=[])
    self.nc.sync.drain()
    self.nc.all_engine_barrier()
    assert self.sems is not None
    popped = self.nc._tile_sem_poison_stack.pop()
    assert popped is self._sem_poison
    self.nc.clear_and_free_semaphores(list(self.sems.allocated().values()))
    self.nc.all_engine_barrier()


_tile.TileContext._drain_and_barrier = _patched_drain_and_barrier

_wnop_counter = [0]


def split_excess_waits(nc, cap=1):
    for f in nc.m.functions:
        for bb in f.blocks:
            insts = bb.instructions
            out = []
            changed = False
            for inst in list(insts):
                si = getattr(inst, "sync_info", None)
                waits = list(si.on_wait) if si is not None else []
                if len(waits) > cap:
                    keep = waits[-cap:]
                    for w in waits[: len(waits) - cap]:
                        _wnop_counter[0] += 1
                        nop = mybir.InstNoOp(
                            name=f"wnop-{_wnop_counter[0]}", ins=[], outs=[]
                        )
                        nop.engine = inst.engine
                        nop.sync_info = mybir.SyncInfo(on_wait=[w], on_update=[])
                        out.append(nop)
                    inst.sync_info = mybir.SyncInfo(
                        on_wait=keep, on_update=list(si.on_update)
                    )
                    changed = True
                out.append(inst)
            if changed:
                insts[:] = out


def dedup_ldweights(nc):
    """The tile lowering emits an explicit InstLdweights before every
    InstMatmult.  Consecutive matmuls that share the stationary operand
    (same AP + tile position) don't need the reload -- the PE keeps its
    weights.  Convert redundant loads into NoOps (keeping their sync info)."""
    n = 0
    for f in nc.m.functions:
        for bb in f.blocks:
            insts = bb.instructions
            last_key = None
            out = []
            changed = False
            for inst in list(insts):
                tn = type(inst).__name__
                if tn == "InstLdweights":
                    key = (
                        str(inst.ins[0]),
                        tuple(inst.tile_position or ()),
                        tuple(inst.tile_size or ()),
                        bool(inst.is_transpose),
                    )
                    if key == last_key:
                        nop = mybir.InstNoOp(name=f"ldwnop-{n}", ins=[], outs=[])
                        n += 1
                        nop.engine = inst.engine
                        si = inst.sync_info
                        if si is not None:
                            nop.sync_info = mybir.SyncInfo(
                                on_wait=list(si.on_wait), on_update=list(si.on_update)
                            )
                        out.append(nop)
                        changed = True
                        continue
                    last_key = key
                elif tn == "InstMatmult":
                    if inst.is_transpose:
                        last_key = None
                out.append(inst)
            if changed:
                insts[:] = out
    return n


# ---------------------------------------------------------------------------
# Device program
# ---------------------------------------------------------------------------


def build_program():
    nc = bass.Bass("TRN2", num_devices=NCORES)
    # z8[r, p, ko, t] = z[r, ko*128 + p, t] as fp8 e4m3 -- the layout the
    # DoubleRow matmul wants ([K=128 partitions, Ko=2, free]).
    z8_in = nc.dram_tensor(
        "z8", [ROWS, 128, FCH, T], dt.float8e4, kind="ExternalInput"
    )
    z_in = nc.dram_tensor("z", [ROWS, F, T], dt.bfloat16, kind="ExternalInput")
    c_in = nc.dram_tensor("c", [ROWS, F, T], dt.bfloat16, kind="ExternalInput")
    # upper-triangle Gram blocks: row block (r*NBLK+tau) holds G[t-block, j]
    # for j in [128*tau, T); the rest of each row is never written.
    g_out = nc.dram_tensor(
        "g", [ROWS * T, T], dt.float16, kind="ExternalOutput"
    )
    # stat[2*r + 0, :] = u (z.c dot), stat[2*r + 1, :] = |c|^2
    stat_out = nc.dram_tensor(
        "stat", [ROWS * 2, T], dt.float32, kind="ExternalOutput"
    )

    with TileContext(nc) as tc:
        with (
            tc.tile_pool(name="io", bufs=2) as io_pool,
            tc.tile_pool(name="work", bufs=2) as work,
            tc.tile_pool(name="outp", bufs=3) as outp,
            tc.tile_pool(name="gram_ps", bufs=6, space="PSUM") as gram_ps,
            tc.tile_pool(name="stat_ps", bufs=2, space="PSUM") as stat_ps,
        ):
            ones16 = io_pool.tile([128, 128], dt.bfloat16, name="ones16")
            nc.vector.memset(ones16[:], 1.0)

            tiles = {}

            def emit_loads(r):
                # fp8 gram operand, loaded in 512-column chunks (on the idle
                # GPSIMD queue) so tau 0's matmuls start as soon as the first
                # chunk lands instead of waiting for the full 1 MB tile.
                z8 = io_pool.tile([128, FCH, T], dt.float8e4, name="z8", tag="z8")
                for ch in range(4):
                    sl = slice(512 * ch, 512 * (ch + 1))
                    nc.gpsimd.dma_start(out=z8[:, :, sl], in_=z8_in[r, :, :, sl])
                zt, ct = [], []
                for j in range(FCH):
                    zj = io_pool.tile(
                        [128, T], dt.bfloat16, name=f"z{j}", tag=f"z{j}"
                    )
                    nc.gpsimd.dma_start(out=zj[:], in_=z_in[r, 128 * j : 128 * (j + 1), :])
                    zt.append(zj)
                for j in range(FCH):
                    cj = io_pool.tile(
                        [128, T], dt.bfloat16, name=f"c{j}", tag=f"c{j}"
                    )
                    nc.gpsimd.dma_start(out=cj[:], in_=c_in[r, 128 * j : 128 * (j + 1), :])
                    ct.append(cj)
                tiles[r] = (z8, zt, ct)

            def stats_pieces(r):
                """Small stat work units, interleaved between gram taus so no
                engine sees a long foreign burst (ACT/DVE are strict FIFO and
                gram PSUM evacuation rides them)."""
                _, zt, ct = tiles[r]
                ut = [
                    work.tile([128, T], dt.bfloat16, name=f"u{j}", tag=f"u{j}")
                    for j in range(FCH)
                ]
                stat_sb = [
                    work.tile([1, T], dt.float32, name=f"st{i}", tag=f"st{i}")
                    for i in range(2)
                ]

                def mul(j):  # u_j = z_j * c_j
                    nc.vector.tensor_tensor(
                        out=ut[j][:], in0=zt[j][:], in1=ct[j][:],
                        op=mybir.AluOpType.mult,
                    )

                def csq(j):  # c_j *= c_j (in place; u reads c first, same FIFO)
                    nc.vector.tensor_tensor(
                        out=ct[j][:], in0=ct[j][:], in1=ct[j][:],
                        op=mybir.AluOpType.mult,
                    )

                def reduce(srcs, stat_idx, quarter):
                    # ones-matmul partition reduction of srcs[j][:, quarter]
                    # into PSUM (sums replicated on every partition); stage
                    # row 0 to SBUF (DMA cannot read PSUM), DMA once the row
                    # is complete.
                    sl = slice(512 * quarter, 512 * (quarter + 1))
                    ps = stat_ps.tile([128, 512], dt.float32, name="sps", tag="sps")
                    for j in range(FCH):
                        nc.tensor.matmul(
                            ps[:], ones16[:], srcs[j][:, sl],
                            start=(j == 0), stop=(j == FCH - 1),
                        )
                    nc.scalar.copy(stat_sb[stat_idx][0:1, sl], ps[0:1, :])
                    if quarter == 3:
                        row = 2 * r + stat_idx
                        nc.sync.dma_start(
                            out=stat_out[row : row + 1, :],
                            in_=stat_sb[stat_idx][0:1, :],
                        )

                yield lambda: mul(0)
                yield lambda: mul(1)
                for q in range(4):
                    yield lambda q=q: reduce(ut, 0, q)
                yield lambda: csq(0)
                yield lambda: csq(1)
                for q in range(4):
                    yield lambda q=q: reduce(ct, 1, q)

            def emit_gram_block(r, tau):
                z8 = tiles[r][0]
                t0 = 128 * tau
                w = T - t0
                nch = (w + 511) // 512
                widths = [min(512, w - 512 * c) for c in range(nch)]
                pts = []
                for c in range(nch):
                    pts.append(
                        gram_ps.tile([128, 512], dt.float32, name="gps", tag="gps")
                    )
                # fp8 DoubleRow: [128, Ko=2, free] operands; the full 256-deep
                # contraction in one pass, 2 MACs/cell/cycle.
                lhsT = z8[:, :, t0 : t0 + 128]
                for c in range(nch):
                    cw = widths[c]
                    c0 = t0 + 512 * c
                    nc.tensor.matmul(
                        pts[c][:, :cw], lhsT, z8[:, :, c0 : c0 + cw],
                        start=True, stop=True,
                        perf_mode=mybir.MatmulPerfMode.DoubleRow,
                    )
                otile = outp.tile([128, T], dt.float16, name="otile", tag="otile")
                for c in range(nch):
                    cw = widths[c]
                    dst = otile[:, 512 * c : 512 * c + cw]
                    if (tau + c) % 2 == 0:
                        nc.scalar.copy(dst, pts[c][:, :cw])
                    else:
                        nc.vector.tensor_copy(dst, pts[c][:, :cw])
                nc.sync.dma_start(
                    out=g_out[(r * NBLK + tau) * 128 : (r * NBLK + tau + 1) * 128, t0:],
                    in_=otile[:, :w],
                )

            emit_loads(0)
            for r in range(ROWS):
                if r + 1 < ROWS:
                    emit_loads(r + 1)
                sid = nc.enter_named_scope(f"gram_r{r}", False)[0]
                pieces = stats_pieces(r)
                for tau in range(NBLK):
                    emit_gram_block(r, tau)
                    if tau >= 1:
                        piece = next(pieces, None)
                        if piece is not None:
                            piece()
                for piece in pieces:
                    piece()
                nc.leave_named_scope(f"gram_r{r}", sid, False)

    dedup_ldweights(nc)
    split_excess_waits(nc)
    return nc


_PROGRAM = None


def _get_program():
    global _PROGRAM
    if _PROGRAM is None:
        _PROGRAM = build_program()
    return _PROGRAM


def kernel(z, c, negative_inds, _trace=False):
    z = np.asarray(z)
    c = np.asarray(c)
    ni = np.asarray(negative_inds)
    assert z.shape == (B, F, T) and c.shape == (B, F, T + 1)

    z16 = np.ascontiguousarray(z.astype(ml_dtypes.bfloat16))
    c16 = np.ascontiguousarray(c[:, :, 1:].astype(ml_dtypes.bfloat16))
    # [B, 128, FCH, T]: z8[b, p, j, t] = z[b, j*128+p, t] (DoubleRow layout)
    z8 = np.ascontiguousarray(
        z.reshape(B, FCH, 128, T).transpose(0, 2, 1, 3).astype(
            ml_dtypes.float8_e4m3fn
        )
    )

    nc = _get_program()
    in_maps = []
    for core in range(NCORES):
        rs = slice(core * ROWS, (core + 1) * ROWS)
        in_maps.append({"z8": z8[rs], "z": z16[rs], "c": c16[rs]})

    res = run_bass_kernel_spmd(nc, in_maps, list(range(NCORES)), trace=_trace)

    # [B, T, T] fp16 raw Gram, upper-triangle blocks valid; [B, 2, T] stats
    g = np.concatenate(
        [res.results[i]["g"].reshape(ROWS, T, T) for i in range(NCORES)], axis=0
    )
    stat = np.concatenate(
        [res.results[i]["stat"].reshape(ROWS, 2, T) for i in range(NCORES)], axis=0
    )  # [B, 2, T]
    u = stat[:, 0, :].astype(np.float64)
    nc2 = stat[:, 1, :].astype(np.float64)

    # host-side unshard: mirror the triangle, normalize, gather (O(output))
    ti = np.arange(T)
    nz2 = np.ascontiguousarray(g[:, ti, ti]).astype(np.float64)  # [B, T] diag
    nz = np.sqrt(nz2)

    n = ni.reshape(B, T, K).astype(np.int64)
    tt = ti[None, :, None]
    valid = n >= (tt // 128) * 128
    rown = np.where(valid, tt, n)
    coln = np.where(valid, n, tt)
    bidx = np.arange(B)[:, None, None]
    graw = g[bidx, rown, coln].astype(np.float64)          # [B, T, K]
    denom = np.maximum(nz[bidx, tt] * nz[bidx, n], EPS)
    neg = (graw / denom) * 2.0

    pos = (u / np.maximum(nz * np.sqrt(nc2), EPS)) * 2.0   # [B, T]

    logits = np.concatenate([pos[:, :, None], neg], axis=2).astype(np.float32)
    out = logits.reshape(B * T, K + 1)
    if _trace:
        return out, res
    return out


if __name__ == "__main__":
    rng = np.random.default_rng(0)
    z = rng.standard_normal((B, F, T), dtype=np.float32)
    c = rng.standard_normal((B, F, T + 1), dtype=np.float32)
    ni = rng.integers(0, T - 1, size=(B, T * K)).astype(np.int64)
    out = kernel(z=z, c=c, negative_inds=ni)
    print("out", out.shape, out.dtype, np.isfinite(out).all())


# revision 26
# speedup vs baseline: 1.0368x; 1.0368x over previous
"""BENDR contrastive-loss kernel for Trainium2 (8 NeuronCores).

Reference computation (see problem): for each (b, t):
  logits[b*T+t, 0]   = cos(z[b,:,t], c[b,:,t+1]) / TEMP
  logits[b*T+t, 1+k] = cos(z[b,:,t], z[b,:,n(b,t,k)]) / TEMP
with n(b,t,k) = negative_inds[b, t*K+k] (row-local), TEMP=0.5.

Strategy: data-parallel over batch (2 rows per core).  Every negative logit
is an entry of the symmetric Gram matrix G = z^T z (z columns = feature
vectors), scaled by 2/(|z_t||z_j|); the norms are G's own diagonal.  So the
device only computes, per batch row:
  - the UPPER-TRIANGLE 128-row blocks of G (raw bf16 z, f32 PSUM) -> fp16
    (tau-th block covers columns [128*tau, T), so ~half the matmuls and
    traffic of the full Gram; the host mirrors lower-triangle lookups),
  - u[t]   = sum_f z[f,t]*c[f,t]   (DVE mult + ones-matmul reduction),
  - nc2[t] = sum_f c[f,t]^2        (same),
shipped as one [1, T] f32 DMA straight out of PSUM partition 0.
The host (pure indexing + O(output) normalize, same spirit as the
baseline's host gather) forms
  neg = 2*G[t,n] / sqrt(G[t,t]*G[n,n]),  pos = 2*u[t] / sqrt(G[t,t]*nc2[t]).

vs. the previous full-Gram kernel this removes the entire on-device
normalization pipeline (reciprocal 62us, input casts, scale mults, scaled
copies) whose DVE/ACT bursts head-blocked PSUM evacuation and let the PE's
HAM clock-gate throttle it to 1.2 GHz.  Here DVE/ACT only carry light
elementwise work + evacuation, and the PE stream is dense.

The gather itself stays on host: GPSIMD indirect_copy measures ~29us per
1024 indices and indirect DMA ~62ns/row -- computing the Gram block on the
PE and shipping fp16 is far cheaper than any on-device gather.
"""

import sys

for _p in ("/opt/trn_rl_repo",):
    if _p not in sys.path:
        sys.path.append(_p)

import numpy as np
import ml_dtypes

import concourse.bass as bass
import concourse.mybir as mybir
from concourse import tile as _tile
from concourse.tile import TileContext
from concourse.bass_utils import run_bass_kernel_spmd

dt = mybir.dt


B, F, T, K = 16, 256, 2048, 20
NCORES = 8
ROWS = B // NCORES          # batch rows per core
NBLK = T // 128             # t-blocks per batch row
FCH = F // 128              # f chunks (partition dim)
EPS = 1e-8

# ---------------------------------------------------------------------------
# Walrus in this container rejects instructions that carry more than one
# semaphore wait ("Too many sync wait commands").  Two shims fix that: the
# tile tail drain gets its waits on single-wait NOPs, and a post-pass splits
# any remaining multi-wait instruction.
# ---------------------------------------------------------------------------


def _patched_drain_and_barrier(self, tick_clock, wait_clock):
    nop0 = self.nc.sync.nop(nofuse=True, hint="tail_wait")
    wait_clock.add_sem_waits(
        nop0.ins, _tile.ScopedClock({None: tick_clock.global_clock})
    )
    si = nop0.ins.sync_info
    if si is not None and len(si.on_wait) > 1:
        waits = list(si.on_wait)
        nop0.ins.sync_info = mybir.SyncInfo(
            on_wait=waits[:1], on_update=list(si.on_update)
        )
        for w in waits[1:]:
            nopi = self.nc.sync.nop(nofuse=True, hint="tail_wait")
            nopi.ins.sync_info = mybir.SyncInfo(on_wait=[w], on_update=[])
    self.nc.sync.drain()
    self.nc.all_engine_barrier()
    assert self.sems is not None
    popped = self.nc._tile_sem_poison_stack.pop()
    assert popped is self._sem_poison
    self.nc.clear_and_free_semaphores(list(self.sems.allocated().values()))
    self.nc.all_engine_barrier()


_tile.TileContext._drain_and_barrier = _patched_drain_and_barrier

_wnop_counter = [0]


def split_excess_waits(nc, cap=1):
    for f in nc.m.functions:
        for bb in f.blocks:
            insts = bb.instructions
            out = []
            changed = False
            for inst in list(insts):
                si = getattr(inst, "sync_info", None)
                waits = list(si.on_wait) if si is not None else []
                if len(waits) > cap:
                    keep = waits[-cap:]
                    for w in waits[: len(waits) - cap]:
                        _wnop_counter[0] += 1
                        nop = mybir.InstNoOp(
                            name=f"wnop-{_wnop_counter[0]}", ins=[], outs=[]
                        )
                        nop.engine = inst.engine
                        nop.sync_info = mybir.SyncInfo(on_wait=[w], on_update=[])
                        out.append(nop)
                    inst.sync_info = mybir.SyncInfo(
                        on_wait=keep, on_update=list(si.on_update)
                    )
                    changed = True
                out.append(inst)
            if changed:
                insts[:] = out


def dedup_ldweights(nc):
    """The tile lowering emits an explicit InstLdweights before every
    InstMatmult.  Consecutive matmuls that share the stationary operand
    (same AP + tile position) don't need the reload -- the PE keeps its
    weights.  Convert redundant loads into NoOps (keeping their sync info)."""
    n = 0
    for f in nc.m.functions:
        for bb in f.blocks:
            insts = bb.instructions
            last_key = None
            out = []
            changed = False
            for inst in list(insts):
                tn = type(inst).__name__
                if tn == "InstLdweights":
                    key = (
                        str(inst.ins[0]),
                        tuple(inst.tile_position or ()),
                        tuple(inst.tile_size or ()),
                        bool(inst.is_transpose),
                    )
                    if key == last_key:
                        nop = mybir.InstNoOp(name=f"ldwnop-{n}", ins=[], outs=[])
                        n += 1
                        nop.engine = inst.engine
                        si = inst.sync_info
                        if si is not None:
                            nop.sync_info = mybir.SyncInfo(
                                on_wait=list(si.on_wait), on_update=list(si.on_update)
                            )
                        out.append(nop)
                        changed = True
                        continue
                    last_key = key
                elif tn == "InstMatmult":
                    if inst.is_transpose:
                        last_key = None
                out.append(inst)
            if changed:
                insts[:] = out
    return n


# ---------------------------------------------------------------------------
# Device program
# ---------------------------------------------------------------------------


def build_program():
    nc = bass.Bass("TRN2", num_devices=NCORES)
    # z8[r, p, ko, t] = z[r, ko*128 + p, t] as fp8 e4m3 -- the layout the
    # DoubleRow matmul wants ([K=128 partitions, Ko=2, free]).
    z8_in = nc.dram_tensor(
        "z8", [ROWS, 128, FCH, T], dt.float8e4, kind="ExternalInput"
    )
    z_in = nc.dram_tensor(
        "z", [ROWS, 128, FCH, T], dt.bfloat16, kind="ExternalInput"
    )
    c_in = nc.dram_tensor(
        "c", [ROWS, 128, FCH, T], dt.bfloat16, kind="ExternalInput"
    )
    # upper-triangle Gram blocks, PARTITION-MAJOR: g[p, r*NBLK+tau, j] =
    # G[128*tau + p, j] (valid for j >= 128*tau).  This layout lets one 3D
    # DMA ship TWO consecutive tau blocks (dims p, tau, j match the SBUF
    # enumeration order), halving the ~700ns-per-DMA trigger cost.
    g_out = nc.dram_tensor(
        "g", [128, ROWS * NBLK, T], dt.float16, kind="ExternalOutput"
    )
    # stat[2*r + 0, :] = u (z.c dot), stat[2*r + 1, :] = |c|^2
    stat_out = nc.dram_tensor(
        "stat", [ROWS * 2, T], dt.float32, kind="ExternalOutput"
    )

    with TileContext(nc) as tc:
        with (
            tc.tile_pool(name="io", bufs=2) as io_pool,
            tc.tile_pool(name="work", bufs=2) as work,
            tc.tile_pool(name="outp", bufs=1) as outp,
            tc.tile_pool(name="gram_ps", bufs=6, space="PSUM") as gram_ps,
            tc.tile_pool(name="stat_ps", bufs=2, space="PSUM") as stat_ps,
        ):
            ones16 = io_pool.tile([128, 128], dt.bfloat16, name="ones16")
            nc.vector.memset(ones16[:], 1.0)

            tiles = {}

            def emit_loads(r):
                # All input loads trigger from the (otherwise idle) GPSIMD
                # queue so the sync queue only carries output DMAs.  Each
                # dma_start costs ~700ns of issuing-engine time, so inputs
                # are 4 big DMAs: z8 in two halves (tau 0 starts after half
                # one), z16 and c16 as single 3D tiles.
                z8 = io_pool.tile([128, FCH, T], dt.float8e4, name="z8", tag="z8")
                for h in range(2):
                    sl = slice(1024 * h, 1024 * (h + 1))
                    nc.gpsimd.dma_start(out=z8[:, :, sl], in_=z8_in[r, :, :, sl])
                z16 = io_pool.tile([128, FCH, T], dt.bfloat16, name="z16", tag="z16")
                nc.gpsimd.dma_start(out=z16[:], in_=z_in[r])
                c16 = io_pool.tile([128, FCH, T], dt.bfloat16, name="c16", tag="c16")
                nc.gpsimd.dma_start(out=c16[:], in_=c_in[r])
                tiles[r] = (z8, z16, c16)

            def stats_pieces(r):
                """Small stat work units, interleaved between gram taus.  The
                elementwise multiplies run on the idle GPSIMD (SBUF-only, no
                PSUM port needed) so the ACT/DVE FIFOs stay almost pure PSUM
                evacuation; the tiny [1,512] stage copies alternate ACT/DVE."""
                _, z16, c16 = tiles[r]
                ut = work.tile([128, FCH, T], dt.bfloat16, name="u", tag="u")
                stat_sb = [
                    work.tile([1, T], dt.float32, name=f"st{i}", tag=f"st{i}")
                    for i in range(2)
                ]

                def mul(j):  # u_j = z_j * c_j  (GPSIMD)
                    nc.gpsimd.tensor_tensor(
                        out=ut[:, j, :], in0=z16[:, j, :], in1=c16[:, j, :],
                        op=mybir.AluOpType.mult,
                    )

                def csq(j):  # c_j *= c_j in place (GPSIMD; u read c earlier
                    # on the same FIFO)
                    nc.gpsimd.tensor_tensor(
                        out=c16[:, j, :], in0=c16[:, j, :], in1=c16[:, j, :],
                        op=mybir.AluOpType.mult,
                    )

                def reduce(src, stat_idx, quarter):
                    # ones-matmul partition reduction of src[:, j, quarter]
                    # into PSUM (sums replicated on every partition); stage
                    # row 0 to SBUF (DMA cannot read PSUM), DMA once the row
                    # is complete.
                    sl = slice(512 * quarter, 512 * (quarter + 1))
                    ps = stat_ps.tile([128, 512], dt.float32, name="sps", tag="sps")
                    for j in range(FCH):
                        nc.tensor.matmul(
                            ps[:], ones16[:], src[:, j, sl],
                            start=(j == 0), stop=(j == FCH - 1),
                        )
                    if (stat_idx * 4 + quarter) % 2 == 0:
                        nc.scalar.copy(stat_sb[stat_idx][0:1, sl], ps[0:1, :])
                    else:
                        nc.vector.tensor_copy(stat_sb[stat_idx][0:1, sl], ps[0:1, :])
                    if quarter == 3:
                        row = 2 * r + stat_idx
                        nc.gpsimd.dma_start(
                            out=stat_out[row : row + 1, :],
                            in_=stat_sb[stat_idx][0:1, :],
                        )

                yield lambda: mul(0)
                yield lambda: mul(1)
                for q in range(4):
                    yield lambda q=q: reduce(ut, 0, q)
                yield lambda: csq(0)
                yield lambda: csq(1)
                for q in range(4):
                    yield lambda q=q: reduce(c16, 1, q)

            # manual ring of 3 pair-otiles ([t-block 2k | t-block 2k+1]; the
            # second block is left-padded 128 junk cols so one 3D DMA covers
            # both blocks with a single column base)
            oring = [
                outp.tile([128, 2, T], dt.float16, name=f"ot{i}", tag=f"ot{i}")
                for i in range(3)
            ]
            evac_flip = [0]

            def emit_gram_tau(r, tau, ot, ko):
                """Matmuls + PSUM evacuation for one tau block into half `ko`
                of the pair otile `ot` (left-padded 128 cols when ko=1)."""
                z8 = tiles[r][0]
                t0 = 128 * tau
                w = T - t0
                nch = (w + 511) // 512
                lhsT = z8[:, :, t0 : t0 + 128]
                pts = []
                for c in range(nch):
                    pts.append(
                        gram_ps.tile([128, 512], dt.float32, name="gps", tag="gps")
                    )
                for c in range(nch):
                    cw = min(512, w - 512 * c)
                    c0 = t0 + 512 * c
                    # fp8 DoubleRow: [128, Ko=2, free] operands; full 256-deep
                    # contraction in one pass, 2 MACs/cell/cycle.
                    nc.tensor.matmul(
                        pts[c][:, :cw], lhsT, z8[:, :, c0 : c0 + cw],
                        start=True, stop=True,
                        perf_mode=mybir.MatmulPerfMode.DoubleRow,
                    )
                pad = 128 * ko
                for c in range(nch):
                    cw = min(512, w - 512 * c)
                    dst = ot[:, ko, pad + 512 * c : pad + 512 * c + cw]
                    if evac_flip[0] % 2 == 0:
                        nc.scalar.copy(dst, pts[c][:, :cw])
                    else:
                        nc.vector.tensor_copy(dst, pts[c][:, :cw])
                    evac_flip[0] += 1

            emit_loads(0)
            for r in range(ROWS):
                if r + 1 < ROWS:
                    emit_loads(r + 1)
                sid = nc.enter_named_scope(f"gram_r{r}", False)[0]
                pieces = stats_pieces(r)
                for pair in range(NBLK // 2):
                    ot = oring[(r * (NBLK // 2) + pair) % 3]
                    emit_gram_tau(r, 2 * pair, ot, 0)
                    emit_gram_tau(r, 2 * pair + 1, ot, 1)
                    wa = T - 256 * pair
                    blk = r * NBLK + 2 * pair
                    nc.sync.dma_start(
                        out=g_out[:, blk : blk + 2, 256 * pair :],
                        in_=ot[:, :, :wa],
                    )
                    if pair >= 1:
                        for _ in range(2):
                            piece = next(pieces, None)
                            if piece is not None:
                                piece()
                for piece in pieces:
                    piece()
                nc.leave_named_scope(f"gram_r{r}", sid, False)

    dedup_ldweights(nc)
    split_excess_waits(nc)
    return nc


_PROGRAM = None


def _get_program():
    global _PROGRAM
    if _PROGRAM is None:
        _PROGRAM = build_program()
    return _PROGRAM


def kernel(z, c, negative_inds, _trace=False):
    z = np.asarray(z)
    c = np.asarray(c)
    ni = np.asarray(negative_inds)
    assert z.shape == (B, F, T) and c.shape == (B, F, T + 1)

    # [B, 128, FCH, T]: x[b, p, j, t] = x[b, j*128+p, t] -- the partition-
    # major layout every SBUF tile uses (and DoubleRow wants for z8).
    zt = z.reshape(B, FCH, 128, T).transpose(0, 2, 1, 3)
    z16 = np.ascontiguousarray(zt.astype(ml_dtypes.bfloat16))
    z8 = np.ascontiguousarray(zt.astype(ml_dtypes.float8_e4m3fn))
    c16 = np.ascontiguousarray(
        c[:, :, 1:].reshape(B, FCH, 128, T).transpose(0, 2, 1, 3).astype(
            ml_dtypes.bfloat16
        )
    )

    nc = _get_program()
    in_maps = []
    for core in range(NCORES):
        rs = slice(core * ROWS, (core + 1) * ROWS)
        in_maps.append({"z8": z8[rs], "z": z16[rs], "c": c16[rs]})

    res = run_bass_kernel_spmd(nc, in_maps, list(range(NCORES)), trace=_trace)

    # [B, T, T] fp16 raw Gram, upper-triangle blocks valid; [B, 2, T] stats.
    # g result arrives partition-major [128, ROWS*NBLK, T].
    g = np.concatenate(
        [
            res.results[i]["g"].transpose(1, 0, 2).reshape(ROWS, T, T)
            for i in range(NCORES)
        ],
        axis=0,
    )
    stat = np.concatenate(
        [res.results[i]["stat"].reshape(ROWS, 2, T) for i in range(NCORES)], axis=0
    )  # [B, 2, T]
    u = stat[:, 0, :].astype(np.float64)
    nc2 = stat[:, 1, :].astype(np.float64)

    # host-side unshard: mirror the triangle, normalize, gather (O(output))
    ti = np.arange(T)
    nz2 = np.ascontiguousarray(g[:, ti, ti]).astype(np.float64)  # [B, T] diag
    nz = np.sqrt(nz2)

    n = ni.reshape(B, T, K).astype(np.int64)
    tt = ti[None, :, None]
    valid = n >= (tt // 128) * 128
    rown = np.where(valid, tt, n)
    coln = np.where(valid, n, tt)
    bidx = np.arange(B)[:, None, None]
    graw = g[bidx, rown, coln].astype(np.float64)          # [B, T, K]
    denom = np.maximum(nz[bidx, tt] * nz[bidx, n], EPS)
    neg = (graw / denom) * 2.0

    pos = (u / np.maximum(nz * np.sqrt(nc2), EPS)) * 2.0   # [B, T]

    logits = np.concatenate([pos[:, :, None], neg], axis=2).astype(np.float32)
    out = logits.reshape(B * T, K + 1)
    if _trace:
        return out, res
    return out


if __name__ == "__main__":
    rng = np.random.default_rng(0)
    z = rng.standard_normal((B, F, T), dtype=np.float32)
    c = rng.standard_normal((B, F, T + 1), dtype=np.float32)
    ni = rng.integers(0, T - 1, size=(B, T * K)).astype(np.int64)
    out = kernel(z=z, c=c, negative_inds=ni)
    print("out", out.shape, out.dtype, np.isfinite(out).all())


# revision 29
# speedup vs baseline: 1.2292x; 1.1855x over previous
"""BENDR contrastive-loss kernel for Trainium2 (8 NeuronCores).

Reference computation (see problem): for each (b, t):
  logits[b*T+t, 0]   = cos(z[b,:,t], c[b,:,t+1]) / TEMP
  logits[b*T+t, 1+k] = cos(z[b,:,t], z[b,:,n(b,t,k)]) / TEMP
with n(b,t,k) = negative_inds[b, t*K+k] (row-local), TEMP=0.5.

Strategy: data-parallel over batch (2 rows per core).  Every negative logit
is an entry of the symmetric Gram matrix G = z^T z (z columns = feature
vectors), scaled by 2/(|z_t||z_j|); the norms are G's own diagonal.  So the
device only computes, per batch row:
  - the UPPER-TRIANGLE 128-row blocks of G (raw bf16 z, f32 PSUM) -> fp16
    (tau-th block covers columns [128*tau, T), so ~half the matmuls and
    traffic of the full Gram; the host mirrors lower-triangle lookups),
  - u[t]   = sum_f z[f,t]*c[f,t]   (DVE mult + ones-matmul reduction),
  - nc2[t] = sum_f c[f,t]^2        (same),
shipped as one [1, T] f32 DMA straight out of PSUM partition 0.
The host (pure indexing + O(output) normalize, same spirit as the
baseline's host gather) forms
  neg = 2*G[t,n] / sqrt(G[t,t]*G[n,n]),  pos = 2*u[t] / sqrt(G[t,t]*nc2[t]).

vs. the previous full-Gram kernel this removes the entire on-device
normalization pipeline (reciprocal 62us, input casts, scale mults, scaled
copies) whose DVE/ACT bursts head-blocked PSUM evacuation and let the PE's
HAM clock-gate throttle it to 1.2 GHz.  Here DVE/ACT only carry light
elementwise work + evacuation, and the PE stream is dense.

The gather itself stays on host: GPSIMD indirect_copy measures ~29us per
1024 indices and indirect DMA ~62ns/row -- computing the Gram block on the
PE and shipping fp16 is far cheaper than any on-device gather.
"""

import sys

for _p in ("/opt/trn_rl_repo",):
    if _p not in sys.path:
        sys.path.append(_p)

import numpy as np
import ml_dtypes

import concourse.bass as bass
import concourse.mybir as mybir
from concourse import tile as _tile
from concourse.tile import TileContext
from concourse.bass_utils import run_bass_kernel_spmd

dt = mybir.dt


B, F, T, K = 16, 256, 2048, 20
NCORES = 8
ROWS = B // NCORES          # batch rows per core
NBLK = T // 128             # t-blocks per batch row
FCH = F // 128              # f chunks (partition dim)
EPS = 1e-8

# ---------------------------------------------------------------------------
# Walrus in this container rejects instructions that carry more than one
# semaphore wait ("Too many sync wait commands").  Two shims fix that: the
# tile tail drain gets its waits on single-wait NOPs, and a post-pass splits
# any remaining multi-wait instruction.
# ---------------------------------------------------------------------------


def _patched_drain_and_barrier(self, tick_clock, wait_clock):
    nop0 = self.nc.sync.nop(nofuse=True, hint="tail_wait")
    wait_clock.add_sem_waits(
        nop0.ins, _tile.ScopedClock({None: tick_clock.global_clock})
    )
    si = nop0.ins.sync_info
    if si is not None and len(si.on_wait) > 1:
        waits = list(si.on_wait)
        nop0.ins.sync_info = mybir.SyncInfo(
            on_wait=waits[:1], on_update=list(si.on_update)
        )
        for w in waits[1:]:
            nopi = self.nc.sync.nop(nofuse=True, hint="tail_wait")
            nopi.ins.sync_info = mybir.SyncInfo(on_wait=[w], on_update=[])
    self.nc.sync.drain()
    self.nc.all_engine_barrier()
    assert self.sems is not None
    popped = self.nc._tile_sem_poison_stack.pop()
    assert popped is self._sem_poison
    self.nc.clear_and_free_semaphores(list(self.sems.allocated().values()))
    self.nc.all_engine_barrier()


_tile.TileContext._drain_and_barrier = _patched_drain_and_barrier

_wnop_counter = [0]


def split_excess_waits(nc, cap=1):
    for f in nc.m.functions:
        for bb in f.blocks:
            insts = bb.instructions
            out = []
            changed = False
            for inst in list(insts):
                si = getattr(inst, "sync_info", None)
                waits = list(si.on_wait) if si is not None else []
                if len(waits) > cap:
                    keep = waits[-cap:]
                    for w in waits[: len(waits) - cap]:
                        _wnop_counter[0] += 1
                        nop = mybir.InstNoOp(
                            name=f"wnop-{_wnop_counter[0]}", ins=[], outs=[]
                        )
                        nop.engine = inst.engine
                        nop.sync_info = mybir.SyncInfo(on_wait=[w], on_update=[])
                        out.append(nop)
                    inst.sync_info = mybir.SyncInfo(
                        on_wait=keep, on_update=list(si.on_update)
                    )
                    changed = True
                out.append(inst)
            if changed:
                insts[:] = out


def dedup_ldweights(nc):
    """The tile lowering emits an explicit InstLdweights before every
    InstMatmult.  Consecutive matmuls that share the stationary operand
    (same AP + tile position) don't need the reload -- the PE keeps its
    weights.  Convert redundant loads into NoOps (keeping their sync info)."""
    n = 0
    for f in nc.m.functions:
        for bb in f.blocks:
            insts = bb.instructions
            last_key = None
            out = []
            changed = False
            for inst in list(insts):
                tn = type(inst).__name__
                if tn == "InstLdweights":
                    key = (
                        str(inst.ins[0]),
                        tuple(inst.tile_position or ()),
                        tuple(inst.tile_size or ()),
                        bool(inst.is_transpose),
                    )
                    if key == last_key:
                        nop = mybir.InstNoOp(name=f"ldwnop-{n}", ins=[], outs=[])
                        n += 1
                        nop.engine = inst.engine
                        si = inst.sync_info
                        if si is not None:
                            nop.sync_info = mybir.SyncInfo(
                                on_wait=list(si.on_wait), on_update=list(si.on_update)
                            )
                        out.append(nop)
                        changed = True
                        continue
                    last_key = key
                elif tn == "InstMatmult":
                    if inst.is_transpose:
                        last_key = None
                out.append(inst)
            if changed:
                insts[:] = out
    return n


# ---------------------------------------------------------------------------
# Device program
# ---------------------------------------------------------------------------


def build_program():
    nc = bass.Bass("TRN2", num_devices=NCORES)
    # z8[r, p, ko, t] = z[r, ko*128 + p, t] as fp8 e4m3 -- the layout the
    # DoubleRow matmul wants ([K=128 partitions, Ko=2, free]).
    z8_in = nc.dram_tensor(
        "z8", [ROWS, 128, FCH, T], dt.float8e4, kind="ExternalInput"
    )
    z_in = nc.dram_tensor(
        "z", [ROWS, 128, FCH, T], dt.bfloat16, kind="ExternalInput"
    )
    c_in = nc.dram_tensor(
        "c", [ROWS, 128, FCH, T], dt.bfloat16, kind="ExternalInput"
    )
    # upper-triangle Gram blocks, PARTITION-MAJOR: g[p, r*NBLK+tau, j] =
    # G[128*tau + p, j] (valid for j >= 128*tau).  This layout lets one 3D
    # DMA ship TWO consecutive tau blocks (dims p, tau, j match the SBUF
    # enumeration order), halving the ~700ns-per-DMA trigger cost.
    g_out = nc.dram_tensor(
        "g", [128, ROWS * NBLK, T], dt.float16, kind="ExternalOutput"
    )
    # stat[2*r + 0, :] = u (z.c dot), stat[2*r + 1, :] = |c|^2
    stat_out = nc.dram_tensor(
        "stat", [ROWS * 2, T], dt.float32, kind="ExternalOutput"
    )

    with TileContext(nc) as tc:
        with (
            tc.tile_pool(name="io", bufs=2) as io_pool,
            tc.tile_pool(name="work", bufs=2) as work,
            tc.tile_pool(name="outp", bufs=1) as outp,
            tc.tile_pool(name="gram_ps", bufs=6, space="PSUM") as gram_ps,
            tc.tile_pool(name="stat_ps", bufs=2, space="PSUM") as stat_ps,
        ):
            ones16 = io_pool.tile([128, 128], dt.bfloat16, name="ones16")
            nc.vector.memset(ones16[:], 1.0)

            tiles = {}

            def emit_loads(r):
                # All input loads trigger from the (otherwise idle) GPSIMD
                # queue so the sync queue only carries output DMAs.  Each
                # dma_start costs ~700ns of issuing-engine time, so inputs
                # are 4 big DMAs: z8 in two halves (tau 0 starts after half
                # one), z16 and c16 as single 3D tiles.
                z8 = io_pool.tile([128, FCH, T], dt.float8e4, name="z8", tag="z8")
                for h in range(2):
                    sl = slice(1024 * h, 1024 * (h + 1))
                    nc.gpsimd.dma_start(out=z8[:, :, sl], in_=z8_in[r, :, :, sl])
                z16 = io_pool.tile([128, FCH, T], dt.bfloat16, name="z16", tag="z16")
                nc.gpsimd.dma_start(out=z16[:], in_=z_in[r])
                c16 = io_pool.tile([128, FCH, T], dt.bfloat16, name="c16", tag="c16")
                nc.gpsimd.dma_start(out=c16[:], in_=c_in[r])
                tiles[r] = (z8, z16, c16)

            def stats_pieces(r):
                """Small stat work units, interleaved between gram taus.  The
                elementwise multiplies run on the idle GPSIMD (SBUF-only, no
                PSUM port needed) so the ACT/DVE FIFOs stay almost pure PSUM
                evacuation; the tiny [1,512] stage copies alternate ACT/DVE."""
                _, z16, c16 = tiles[r]
                ut = work.tile([128, FCH, T], dt.bfloat16, name="u", tag="u")
                stat_sb = [
                    work.tile([1, T], dt.float32, name=f"st{i}", tag=f"st{i}")
                    for i in range(2)
                ]

                def mul(j):  # u_j = z_j * c_j  (GPSIMD)
                    nc.gpsimd.tensor_tensor(
                        out=ut[:, j, :], in0=z16[:, j, :], in1=c16[:, j, :],
                        op=mybir.AluOpType.mult,
                    )

                def csq(j):  # c_j *= c_j in place (GPSIMD; u read c earlier
                    # on the same FIFO)
                    nc.gpsimd.tensor_tensor(
                        out=c16[:, j, :], in0=c16[:, j, :], in1=c16[:, j, :],
                        op=mybir.AluOpType.mult,
                    )

                def reduce(src, stat_idx, quarter):
                    # ones-matmul partition reduction of src[:, j, quarter]
                    # into PSUM (sums replicated on every partition); stage
                    # row 0 to SBUF (DMA cannot read PSUM), DMA once the row
                    # is complete.
                    sl = slice(512 * quarter, 512 * (quarter + 1))
                    ps = stat_ps.tile([128, 512], dt.float32, name="sps", tag="sps")
                    for j in range(FCH):
                        nc.tensor.matmul(
                            ps[:], ones16[:], src[:, j, sl],
                            start=(j == 0), stop=(j == FCH - 1),
                        )
                    if (stat_idx * 4 + quarter) % 2 == 0:
                        nc.scalar.copy(stat_sb[stat_idx][0:1, sl], ps[0:1, :])
                    else:
                        nc.vector.tensor_copy(stat_sb[stat_idx][0:1, sl], ps[0:1, :])
                    if quarter == 3:
                        # sync queue, NOT gpsimd: a dependency-gated trigger
                        # would head-block gpsimd's multiply FIFO for ~20us.
                        row = 2 * r + stat_idx
                        nc.sync.dma_start(
                            out=stat_out[row : row + 1, :],
                            in_=stat_sb[stat_idx][0:1, :],
                        )

                yield lambda: mul(0)
                yield lambda: mul(1)
                for q in range(4):
                    yield lambda q=q: reduce(ut, 0, q)
                yield lambda: csq(0)
                yield lambda: csq(1)
                for q in range(4):
                    yield lambda q=q: reduce(c16, 1, q)

            # manual ring of 6 pair-otiles ([t-block 2k | t-block 2k+1]; the
            # second block is left-padded 128 junk cols so one 3D DMA covers
            # both blocks with a single column base).  6 deep because the
            # early pair DMAs are ~2MB / ~5us: with only 3 slots the
            # evacuation (and then the PE, via the PSUM ring) stalls on the
            # write-after-read of a slot still being shipped out.
            NOR = 6
            oring = [
                outp.tile([128, 2, T], dt.float16, name=f"ot{i}", tag=f"ot{i}")
                for i in range(NOR)
            ]
            evac_flip = [0]

            def emit_gram_tau(r, tau, ot, ko):
                """Matmuls + PSUM evacuation for one tau block into half `ko`
                of the pair otile `ot` (left-padded 128 cols when ko=1)."""
                z8 = tiles[r][0]
                t0 = 128 * tau
                w = T - t0
                nch = (w + 511) // 512
                lhsT = z8[:, :, t0 : t0 + 128]
                pts = []
                for c in range(nch):
                    pts.append(
                        gram_ps.tile([128, 512], dt.float32, name="gps", tag="gps")
                    )
                for c in range(nch):
                    cw = min(512, w - 512 * c)
                    c0 = t0 + 512 * c
                    # fp8 DoubleRow: [128, Ko=2, free] operands; full 256-deep
                    # contraction in one pass, 2 MACs/cell/cycle.
                    nc.tensor.matmul(
                        pts[c][:, :cw], lhsT, z8[:, :, c0 : c0 + cw],
                        start=True, stop=True,
                        perf_mode=mybir.MatmulPerfMode.DoubleRow,
                    )
                pad = 128 * ko
                for c in range(nch):
                    cw = min(512, w - 512 * c)
                    dst = ot[:, ko, pad + 512 * c : pad + 512 * c + cw]
                    if evac_flip[0] % 2 == 0:
                        nc.scalar.copy(dst, pts[c][:, :cw])
                    else:
                        nc.vector.tensor_copy(dst, pts[c][:, :cw])
                    evac_flip[0] += 1

            emit_loads(0)
            for r in range(ROWS):
                if r + 1 < ROWS:
                    emit_loads(r + 1)
                sid = nc.enter_named_scope(f"gram_r{r}", False)[0]
                pieces = stats_pieces(r)
                for pair in range(NBLK // 2):
                    ot = oring[(r * (NBLK // 2) + pair) % NOR]
                    emit_gram_tau(r, 2 * pair, ot, 0)
                    emit_gram_tau(r, 2 * pair + 1, ot, 1)
                    wa = T - 256 * pair
                    blk = r * NBLK + 2 * pair
                    nc.sync.dma_start(
                        out=g_out[:, blk : blk + 2, 256 * pair :],
                        in_=ot[:, :, :wa],
                    )
                    if pair >= 1:
                        for _ in range(2):
                            piece = next(pieces, None)
                            if piece is not None:
                                piece()
                for piece in pieces:
                    piece()
                nc.leave_named_scope(f"gram_r{r}", sid, False)

    dedup_ldweights(nc)
    split_excess_waits(nc)
    return nc


_PROGRAM = None


def _get_program():
    global _PROGRAM
    if _PROGRAM is None:
        _PROGRAM = build_program()
    return _PROGRAM


def kernel(z, c, negative_inds, _trace=False):
    z = np.asarray(z)
    c = np.asarray(c)
    ni = np.asarray(negative_inds)
    assert z.shape == (B, F, T) and c.shape == (B, F, T + 1)

    # [B, 128, FCH, T]: x[b, p, j, t] = x[b, j*128+p, t] -- the partition-
    # major layout every SBUF tile uses (and DoubleRow wants for z8).
    zt = z.reshape(B, FCH, 128, T).transpose(0, 2, 1, 3)
    z16 = np.ascontiguousarray(zt.astype(ml_dtypes.bfloat16))
    z8 = np.ascontiguousarray(zt.astype(ml_dtypes.float8_e4m3fn))
    c16 = np.ascontiguousarray(
        c[:, :, 1:].reshape(B, FCH, 128, T).transpose(0, 2, 1, 3).astype(
            ml_dtypes.bfloat16
        )
    )

    nc = _get_program()
    in_maps = []
    for core in range(NCORES):
        rs = slice(core * ROWS, (core + 1) * ROWS)
        in_maps.append({"z8": z8[rs], "z": z16[rs], "c": c16[rs]})

    res = run_bass_kernel_spmd(nc, in_maps, list(range(NCORES)), trace=_trace)

    # [B, T, T] fp16 raw Gram, upper-triangle blocks valid; [B, 2, T] stats.
    # g result arrives partition-major [128, ROWS*NBLK, T].
    g = np.concatenate(
        [
            res.results[i]["g"].transpose(1, 0, 2).reshape(ROWS, T, T)
            for i in range(NCORES)
        ],
        axis=0,
    )
    stat = np.concatenate(
        [res.results[i]["stat"].reshape(ROWS, 2, T) for i in range(NCORES)], axis=0
    )  # [B, 2, T]
    u = stat[:, 0, :].astype(np.float64)
    nc2 = stat[:, 1, :].astype(np.float64)

    # host-side unshard: mirror the triangle, normalize, gather (O(output))
    ti = np.arange(T)
    nz2 = np.ascontiguousarray(g[:, ti, ti]).astype(np.float64)  # [B, T] diag
    nz = np.sqrt(nz2)

    n = ni.reshape(B, T, K).astype(np.int64)
    tt = ti[None, :, None]
    valid = n >= (tt // 128) * 128
    rown = np.where(valid, tt, n)
    coln = np.where(valid, n, tt)
    bidx = np.arange(B)[:, None, None]
    graw = g[bidx, rown, coln].astype(np.float64)          # [B, T, K]
    denom = np.maximum(nz[bidx, tt] * nz[bidx, n], EPS)
    neg = (graw / denom) * 2.0

    pos = (u / np.maximum(nz * np.sqrt(nc2), EPS)) * 2.0   # [B, T]

    logits = np.concatenate([pos[:, :, None], neg], axis=2).astype(np.float32)
    out = logits.reshape(B * T, K + 1)
    if _trace:
        return out, res
    return out


if __name__ == "__main__":
    rng = np.random.default_rng(0)
    z = rng.standard_normal((B, F, T), dtype=np.float32)
    c = rng.standard_normal((B, F, T + 1), dtype=np.float32)
    ni = rng.integers(0, T - 1, size=(B, T * K)).astype(np.int64)
    out = kernel(z=z, c=c, negative_inds=ni)
    print("out", out.shape, out.dtype, np.isfinite(out).all())
